# revision 8
# baseline (speedup 1.0000x reference)
"""Multi-head attention (B=4, S=2048, D=1024, 16 heads x 64) on 8 trn2 cores.

Sharding: core c handles batch b = c//2 and head-group hg = c%2 (8 heads each,
i.e. columns hg*512:(hg+1)*512 of Wq/Wk/Wv and rows of Wo).  Each core returns
a partial output [S, D]; the host sums the two partials per batch and adds bo.

v2 design (vs the serial-matmul baseline):
  * all matmul operands bf16 (inputs converted host-side) -- same PE rate as
    f32r but enables FWL weight loads and halves SBUF/DMA.
  * phase 2 packs the array: scores run as 64x128 row-tile pairs (two heads
    co-execute, rows 0:63 / 64:127), PV runs as 128x64 col-tile pairs (two
    heads' z into one PSUM bank at partitions 0:63/64:127), and the softmax
    denominators are 128x32 4-way col-tiled ones-matmuls into one bank.
  * exp is split between ScalarE (exact, every EXP_SCALAR_EVERY-th unit) and
    DVE (Schraudolph bf16-bit trick: probs = bitcast(i16(s*A + B)), ~2% rms,
    bias cancels in softmax normalization) so neither engine is the wall.
  * phase 3 (out-proj) for chunk c runs right after chunk c's attention,
    overlapping the next chunk's exp/DVE work.

build_nc(reps=N) wraps the body in a hardware For_i loop: the whole kernel
re-executes N times per dispatch, letting the test harness difference two rep
counts to measure on-device time despite the ~0.8 ms axon dispatch floor.
"""

import numpy as np
import ml_dtypes

import concourse.bass as bass
import concourse.tile as tile
from concourse import bacc, mybir
from concourse.bass_utils import run_bass_kernel_spmd

F32 = mybir.dt.float32
BF16 = mybir.dt.bfloat16
I16 = mybir.dt.int16
ACT = mybir.ActivationFunctionType
ALU = mybir.AluOpType

D = 1024          # d_model
HH = 512          # heads-per-core * head_dim = 8 * 64
HD = 64           # head dim
B, S_FULL = 4, 2048
N_CORES = 8

LOG2E = 1.4426950408889634
SCHRAUD_A = 0.125 * LOG2E * 128.0          # scores scale 1/8 folded in
SCHRAUD_B = 127.0 * 128.0 - 5.5            # C=5.5: max rel 3.3%, rms 2.1%
EXP_DVE_EVERY = 4    # every Nth exp unit approximated on DVE (Schraudolph);
                     # the rest exact on ScalarE.  Error scales ~sqrt(1/N).


def build_nc(S=S_FULL, reps=1):
    use_dn_mm = True
    nc = bacc.Bacc("TRN2", target_bir_lowering=False, debug=False,
                   dynamic_dma_scratch_size=2048)

    xqT = nc.dram_tensor("xqT", [D, S], BF16, kind="ExternalInput").ap()
    xkT = nc.dram_tensor("xkT", [D, S], BF16, kind="ExternalInput").ap()
    xvT = nc.dram_tensor("xvT", [D, S], BF16, kind="ExternalInput").ap()
    wq = nc.dram_tensor("wq", [D, HH], BF16, kind="ExternalInput").ap()
    wk = nc.dram_tensor("wk", [D, HH], BF16, kind="ExternalInput").ap()
    wv = nc.dram_tensor("wv", [D, HH], BF16, kind="ExternalInput").ap()
    wo = nc.dram_tensor("wo", [HH, D], BF16, kind="ExternalInput").ap()
    bq = nc.dram_tensor("bq", [HH], F32, kind="ExternalInput").ap()
    bk = nc.dram_tensor("bk", [HH], F32, kind="ExternalInput").ap()
    bv = nc.dram_tensor("bv", [HH], F32, kind="ExternalInput").ap()
    out = nc.dram_tensor("out", [S, D], F32, kind="ExternalOutput").ap()

    NT = S // 512        # 512-token chunks
    NSK = S // 128       # 128-token key tiles
    NKT = D // 128       # 128-wide d_model tiles
    NKB = HH // 128      # 128-wide hidden tiles (head pairs)

    with tile.TileContext(nc) as tc:
        from contextlib import ExitStack

        rep_loop = tc.For_i(0, reps, 1) if reps > 1 else None
        if rep_loop is not None:
            rep_loop.__enter__()

        with ExitStack() as ctx:
            persist = ctx.enter_context(tc.tile_pool(name="persist", bufs=1))
            qt_sb = persist.tile([128, NKB, S], BF16, tag="qt")
            kt_sb = persist.tile([128, NKB, S], BF16, tag="kt")
            vb_sb = persist.tile([128, NSK, HH], BF16, tag="vb")
            zt_sb = persist.tile([128, NKB, S], BF16, tag="zt")
            wo_sb = persist.tile([128, NKB, D], BF16, tag="wo")
            bq_sb = persist.tile([128, NKB], F32, tag="bq")
            bk_sb = persist.tile([128, NKB], F32, tag="bk")
            bvb_sb = persist.tile([128, HH], F32, tag="bvb")
            ones_sb = persist.tile([128, 1], BF16, tag="ones")

            nc.sync.dma_start(out=bq_sb, in_=bq.rearrange("(kb p) -> p kb", p=128))
            nc.sync.dma_start(out=bk_sb, in_=bk.rearrange("(kb p) -> p kb", p=128))
            bv_bcast_in = bass.AP(tensor=bv.tensor, offset=bv.offset,
                                  ap=[[0, 128], [1, HH]])
            nc.sync.dma_start(out=bvb_sb, in_=bv_bcast_in)
            nc.vector.memset(ones_sb, 1.0)
            # preload the exp ACT table before the pipeline needs it
            warm = persist.tile([1, 1], BF16, tag="warm")
            nc.scalar.activation(warm, ones_sb[0:1, :], ACT.Exp, scale=1.0)

            # ---------------- phase 1: projections ----------------
            with ExitStack() as c1:
                wpool = c1.enter_context(tc.tile_pool(name="wpool", bufs=2))
                xpool = c1.enter_context(tc.tile_pool(name="xpool", bufs=3))
                p1 = c1.enter_context(tc.tile_pool(name="p1", bufs=4, space="PSUM"))

                # K first: scores for chunk c need full KT but only chunk c
                # of QT, so attention starts while Q still streams.
                for (xT, w_dram, dst, bias) in ((xkT, wk, kt_sb, bk_sb),
                                                (xqT, wq, qt_sb, bq_sb)):
                    w_sb = wpool.tile([128, NKT, HH], BF16, tag="w",
                                      name=f"w_{dst.name}")
                    nc.sync.dma_start(out=w_sb,
                                      in_=w_dram.rearrange("(kt p) n -> p kt n", p=128))
                    for t in range(NT):
                        xt = xpool.tile([128, NKT, 512], BF16, tag="xt")
                        nc.sync.dma_start(
                            out=xt,
                            in_=xT.rearrange("(kt p) s -> p kt s", p=128)[:, :, t * 512:(t + 1) * 512])
                        for kb in range(NKB):
                            ps = p1.tile([128, 512], F32, tag="ps1")
                            for kt in range(NKT):
                                nc.tensor.matmul(
                                    ps,
                                    lhsT=w_sb[:, kt, kb * 128:(kb + 1) * 128],
                                    rhs=xt[:, kt, :],
                                    start=(kt == 0), stop=(kt == NKT - 1))
                            nc.vector.tensor_scalar_add(
                                dst[:, kb, t * 512:(t + 1) * 512], ps,
                                bias[:, kb:kb + 1])

                # V pass: natural [tokens, hidden]
                wv_sb = wpool.tile([128, NKT, HH], BF16, tag="w", name="w_v")
                nc.sync.dma_start(out=wv_sb,
                                  in_=wv.rearrange("(kt p) n -> p kt n", p=128))
                for t in range(NT):
                    xt = xpool.tile([128, NKT, 512], BF16, tag="xt")
                    nc.sync.dma_start(
                        out=xt,
                        in_=xvT.rearrange("(kt p) s -> p kt s", p=128)[:, :, t * 512:(t + 1) * 512])
                    for m in range(4):
                        ps = p1.tile([128, 512], F32, tag="ps1")
                        for kt in range(NKT):
                            nc.tensor.matmul(
                                ps,
                                lhsT=xt[:, kt, m * 128:(m + 1) * 128],
                                rhs=wv_sb[:, kt, :],
                                start=(kt == 0), stop=(kt == NKT - 1))
                        nc.vector.tensor_add(vb_sb[:, t * 4 + m, :], ps, bvb_sb)

            # wo loads here to keep the early DMA window clear for xkT/xqT
            nc.sync.dma_start(out=wo_sb, in_=wo.rearrange("(hb p) n -> p hb n", p=128))

            # ---------------- phase 2 + 3 ----------------
            with ExitStack() as c2:
                sppool = c2.enter_context(tc.tile_pool(name="sppool", bufs=2, space="PSUM"))
                zqpool = c2.enter_context(tc.tile_pool(name="zqpool", bufs=1, space="PSUM"))
                dnpool = c2.enter_context(tc.tile_pool(name="dnpool", bufs=1, space="PSUM"))
                p3pool = c2.enter_context(tc.tile_pool(name="p3pool", bufs=1, space="PSUM"))
                ptpool = c2.enter_context(tc.tile_pool(name="ptpool", bufs=6))
                npool = c2.enter_context(tc.tile_pool(name="npool", bufs=2))
                opool = c2.enter_context(tc.tile_pool(name="opool", bufs=2))

                exp_unit = 0
                for c in range(NT):
                    for qd in range(2):
                        zps = [zqpool.tile([128, 512], F32, tag=f"z{p}",
                                           name=f"zps{p}_{qd}_{c}")
                               for p in range(2)]
                        if use_dn_mm:
                            dn = dnpool.tile([128, 512], F32, tag="dn",
                                             name=f"dn_{qd}_{c}")
                        pts = {}
                        for sk in range(NSK + 1):
                            if sk < NSK:
                                for p in range(2):
                                    kb = 2 * qd + p
                                    spt = sppool.tile([128, 2, 512], F32, tag="sp")
                                    for d in range(2):
                                        nc.tensor.matmul(
                                            spt[:, d, :],
                                            lhsT=kt_sb[d * 64:(d + 1) * 64, kb,
                                                       sk * 128:(sk + 1) * 128],
                                            rhs=qt_sb[d * 64:(d + 1) * 64, kb,
                                                      c * 512:(c + 1) * 512],
                                            start=True, stop=True)
                                    pt = ptpool.tile([128, 2, 512], BF16, tag="pt")
                                    if exp_unit % EXP_DVE_EVERY == EXP_DVE_EVERY - 1:
                                        nc.vector.tensor_scalar(
                                            pt.bitcast(I16), spt,
                                            SCHRAUD_A, SCHRAUD_B,
                                            ALU.mult, ALU.add)
                                    else:
                                        nc.scalar.activation(pt, spt, ACT.Exp,
                                                             scale=0.125)
                                    exp_unit += 1
                                    pts[(p, sk)] = pt
                            if sk >= 1:
                                j = sk - 1
                                for p in range(2):
                                    kb = 2 * qd + p
                                    pt = pts[(p, j)]
                                    for d in range(2):
                                        hh = 2 * kb + d
                                        nc.tensor.matmul(
                                            zps[p][d * 64:(d + 1) * 64, :],
                                            lhsT=vb_sb[:, j, hh * 64:(hh + 1) * 64],
                                            rhs=pt[:, d, :],
                                            start=(j == 0), stop=(j == NSK - 1))
                                if use_dn_mm:
                                    for jj in range(4):
                                        p, d = jj // 2, jj % 2
                                        nc.tensor.matmul(
                                            dn[jj * 32:jj * 32 + 1, :],
                                            lhsT=ones_sb,
                                            rhs=pts[(p, j)][:, d, :],
                                            start=(j == 0), stop=(j == NSK - 1),
                                            tile_position=(0, jj * 32))
                                for p in range(2):
                                    del pts[(p, j)]

                        # normalize quad: z / denominator -> zt bf16
                        for jj in range(4):
                            p, d = jj // 2, jj % 2
                            kb = 2 * qd + p
                            dcp = npool.tile([1, 512], F32, tag="dcp")
                            nc.vector.tensor_copy(dcp, dn[jj * 32:jj * 32 + 1, :])
                            rc = npool.tile([1, 512], F32, tag="rc")
                            nc.vector.reciprocal_approx_fast(rc, dcp)
                            bc = npool.tile([HD, 512], F32, tag="bc")
                            nc.gpsimd.partition_broadcast(bc, rc, channels=HD)
                            nc.vector.tensor_mul(
                                zt_sb[d * 64:(d + 1) * 64, kb, c * 512:(c + 1) * 512],
                                zps[p][d * 64:(d + 1) * 64, :], bc)

                    # phase 3 for chunk c
                    for t in range(4):
                        tok = c * 4 + t
                        os_t = opool.tile([128, D], F32, tag="os")
                        for n in range(2):
                            po = p3pool.tile([128, 512], F32, tag="po")
                            for hb in range(NKB):
                                nc.tensor.matmul(
                                    po,
                                    lhsT=zt_sb[:, hb, tok * 128:(tok + 1) * 128],
                                    rhs=wo_sb[:, hb, n * 512:(n + 1) * 512],
                                    start=(hb == 0), stop=(hb == NKB - 1))
                            nc.vector.tensor_copy(os_t[:, n * 512:(n + 1) * 512],
                                                  po)
                        nc.sync.dma_start(out=out[tok * 128:(tok + 1) * 128, :],
                                          in_=os_t)

        if rep_loop is not None:
            rep_loop.__exit__(None, None, None)

    nc.compile()
    return nc


_NC_CACHE = {}


def _get_nc(S=S_FULL, reps=1):
    key = (S, reps)
    if key not in _NC_CACHE:
        _NC_CACHE[key] = build_nc(S, reps=reps)
    return _NC_CACHE[key]


def make_in_maps(query, key, value, Wq, bq, Wk, bk, Wv, bv, Wo, bo):
    """Shard full inputs into 8 per-core input dicts (bf16 operands)."""
    bf = lambda a: np.ascontiguousarray(np.asarray(a, dtype=np.float32)).astype(ml_dtypes.bfloat16)
    f32 = lambda a: np.ascontiguousarray(np.asarray(a, dtype=np.float32))
    in_maps = []
    for core in range(N_CORES):
        b, hg = core // 2, core % 2
        sl = slice(hg * HH, (hg + 1) * HH)
        in_maps.append({
            "xqT": bf(np.asarray(query)[b].T),
            "xkT": bf(np.asarray(key)[b].T),
            "xvT": bf(np.asarray(value)[b].T),
            "wq": bf(np.asarray(Wq)[:, sl]),
            "wk": bf(np.asarray(Wk)[:, sl]),
            "wv": bf(np.asarray(Wv)[:, sl]),
            "wo": bf(np.asarray(Wo)[sl, :]),
            "bq": f32(np.asarray(bq)[sl]),
            "bk": f32(np.asarray(bk)[sl]),
            "bv": f32(np.asarray(bv)[sl]),
        })
    return in_maps


def kernel(query, key, value, Wq, bq, Wk, bk, Wv, bv, Wo, bo, **run_kwargs):
    nc = _get_nc(S_FULL)
    in_maps = make_in_maps(query, key, value, Wq, bq, Wk, bk, Wv, bv, Wo, bo)
    res = run_bass_kernel_spmd(nc, in_maps, core_ids=list(range(N_CORES)),
                               **run_kwargs)
    bo_np = np.asarray(bo, dtype=np.float32)
    outs = [np.asarray(r["out"], dtype=np.float32) for r in res.results]
    full = np.stack([outs[2 * b] + outs[2 * b + 1] + bo_np for b in range(B)])
    return full.astype(np.float32)
